# revision 46
# baseline (speedup 1.0000x reference)
"""Bicubic sparse grid_sample (InterpolateSparse2d) for Trainium2.

Strategy: data-parallel over batch (8 batches -> 8 NeuronCores).
Per core:
  - compute clamped 4x4 patch window start (sx, sy) per point on-device
  - SWDGE dma_gather of the 4x4x64 patches; GpSimd descriptor generation
    (~8ns/desc on 2 Q7 cores) is the bottleneck, so descriptors per point
    are cut from 4 to 2 for most tiles via xC, a row-pair-interleaved HBM
    copy of x (xC[r] = [x[r] | x[r+160]]) built on the fly by HBM->HBM
    DMAs on the otherwise-idle scalar HWDGE queue: one 2KB element at xC
    row sy*160+sx covers patch rows (sy, sy+1), row +320 covers (sy+2,
    sy+3). Early tiles (phase 1) gather 4-desc straight from x while the
    build is in flight; later tiles (phase 2) gather 2-desc from xC.
  - cubic weights W(d) evaluated branchlessly per fetched slot; OOB taps
    get |d|>=2 -> weight 0 (matches zeros-padding + ref masking)
  - per-partition-scalar multiply-accumulate reduce on VectorE
    (scalar_tensor_tensor only: tensor_scalar with a PTR scalar hits a
    sporadic 17-26us HW slow path)

Point-to-partition mapping is n = p*32 + t (32 consecutive points per
partition) so the pos loads (128-land for weights, 16-land for gather
indices) and the output writeback are all contiguous-chunk DMAs.  Gather
chunks are small-first (quick first MAC) and tiny-last (short tail).
"""

import numpy as np

import concourse.bacc as bacc
import concourse.mybir as mybir
import concourse.tile as tile
from concourse import bass_utils
from concourse.ap import AP

F32 = mybir.dt.float32
I32 = mybir.dt.int32
I16 = mybir.dt.int16
ALU = mybir.AluOpType
AF = mybir.ActivationFunctionType

B, Hf, Wf, C = 8, 160, 160, 64
N = 4096
H, W = 1280, 1280
A = -0.75
SCALE = float(Wf) / float(W - 1)  # == Hf/(H-1); maps pixel coords -> feature coords
P = 128
NT = N // P          # 32 point-tiles of 128
# phase 1: 4-desc gathers straight from x while xC builds in background;
# phase 2: 2-desc gathers from the row-pair-interleaved xC copy.
CHUNKS1 = [1, 2, 3, 4]        # tiles per gather, phase 1
CHUNKS2 = [4, 5, 5, 5, 2, 1]  # tiles per gather, phase 2
NPIX = Hf * Wf       # 25600
# gather source AP row count: last valid row start is 156*160+156+480 = 25596,
# and (rows-1)*64 + 256 must stay inside the x tensor (25600*64 elements)
SRC_ROWS = NPIX - 3  # 25597
XC_ROWS = 25440      # xC rows actually referenced: r0+323 <= 25439
MAXT = 8


def build_nc(num_devices: int, iters: int = 1):
    nc = bacc.Bacc(
        "TRN2", target_bir_lowering=False, debug=False,
        enable_asserts=False, num_devices=num_devices,
    )
    x_d = nc.dram_tensor("x", [NPIX, C], F32, kind="ExternalInput").ap()
    pos_d = nc.dram_tensor("pos", [N, 2], F32, kind="ExternalInput").ap()
    out_d = nc.dram_tensor("out", [N, C], F32, kind="ExternalOutput").ap()

    x_src = AP(tensor=x_d.tensor, offset=0, ap=[[C, SRC_ROWS], [1, 4 * C]])

    with tile.TileContext(nc) as tc:
        with tc.tile_pool(name="sbuf", bufs=1) as pool, \
             tc.tile_pool(name="dpool", bufs=1, space="DRAM") as dpool, \
             tc.tile_pool(name="gpool", bufs=4) as gpool:
            v = nc.vector

            # ------- xC: row-pair interleave in HBM ------------------------
            # xC[r] = [x[r] | x[r+160]]; a 4-row gather element at xC row r0
            # covers the 4x4 patch rows (sy, sy+1); r0+320 covers (sy+2, sy+3).
            # Built below (after the idx-path DMAs are queued) by pure
            # HBM->HBM DMAs on the sync/scalar HWDGE queues; the 12720-desc
            # issues overlap the phase-1 gathers.
            xc = dpool.tile([NPIX, 2 * C], F32)
            xc_t = xc[:].tensor
            xc_src = AP(tensor=xc_t, offset=0,
                        ap=[[2 * C, XC_ROWS - 3], [1, 4 * 2 * C]])

            # ---------------- pos load: pos128[p, s*2+c] = pos[p*32+s, c] ----
            # contiguous 256B per partition
            pos128 = pool.tile([P, NT * 2], F32)
            nc.sync.dma_start(
                out=pos128[:].rearrange("p (s c) -> p s c", c=2),
                in_=pos_d.rearrange("(p s) c -> p s c", p=P),
            )

            # ---------------- gather indices (16-partition land) -------------
            # The idx tile must live in partitions 0..31 as [j, col] with
            # col = (slot)*8 + ph for point p = ph*16+j.  Computing the chain
            # directly in 16-partition land (from a second pos load laid out
            # [j, (h, t, c)]) avoids any 128->16 transpose DMAs, keeping the
            # early HWDGE DMA count below the sem-pool size.
            pos16 = pool.tile([P, 512], F32)
            for jb in (0, 16):
                nc.sync.dma_start(
                    out=pos16[jb:jb + 16].rearrange("j (h t c) -> j h t c",
                                                    h=8, c=2),
                    in_=pos_d.rearrange("(h j t) c -> j h t c", h=8, j=16),
                )
            ixy16 = pool.tile([P, 512], F32)
            v.tensor_scalar(out=ixy16[0:32], in0=pos16[0:32], scalar1=SCALE,
                            scalar2=-0.5, op0=ALU.mult, op1=ALU.add)
            r16 = pool.tile([P, 512], I32)
            v.tensor_copy(out=r16[0:32], in_=ixy16[0:32])
            rf16 = pool.tile([P, 512], F32)
            v.tensor_copy(out=rf16[0:32], in_=r16[0:32])
            gt16 = pool.tile([P, 512], F32)
            v.tensor_tensor(out=gt16[0:32], in0=rf16[0:32],
                            in1=ixy16[0:32], op=ALU.is_gt)
            fl16 = pool.tile([P, 512], F32)
            v.tensor_tensor(out=fl16[0:32], in0=rf16[0:32],
                            in1=gt16[0:32], op=ALU.subtract)
            sf16 = pool.tile([P, 512], F32)
            v.tensor_scalar(out=sf16[0:32], in0=fl16[0:32], scalar1=-1.0,
                            scalar2=0.0, op0=ALU.add, op1=ALU.max)
            v.tensor_scalar(out=sf16[0:32], in0=sf16[0:32],
                            scalar1=float(Wf - 4), scalar2=None, op0=ALU.min)
            # base16[j, h*32+t] = sy*160 + sx for point (h*16+j)*32 + t
            base16 = pool.tile([P, NT * 8], F32)
            sxy16 = sf16[0:32].rearrange("j (h t c) -> j (h t) c", h=8, c=2)
            v.tensor_scalar(out=base16[0:32], in0=sxy16[:, :, 1],
                                    scalar1=float(Wf), scalar2=None,
                                    op0=ALU.mult)
            v.tensor_tensor(out=base16[0:32], in0=base16[0:32],
                                    in1=sxy16[:, :, 0], op=ALU.add)
            # phase-1 idx cols [0, NT*32): [j, t*32 + it*8 + ph] = base + 160*it
            # phase-2 idx cols [NT*32, NT*48): [j, t*16 + d*8 + ph] = base + 320*d
            idxf16 = pool.tile([P, NT * 48], F32)
            bv = base16[0:32].rearrange("j (h t) -> j t h", t=NT)
            for it in range(4):
                outv = idxf16[0:32, :NT * 32].rearrange(
                    "j (t i h) -> j t i h", i=4, h=8)[:, :, it, :]
                v.tensor_scalar(out=outv, in0=bv, scalar1=float(it * Wf),
                                scalar2=None, op0=ALU.add)
            for d in range(2):
                outv = idxf16[0:32, NT * 32:].rearrange(
                    "j (t i h) -> j t i h", i=2, h=8)[:, :, d, :]
                v.tensor_scalar(out=outv, in0=bv, scalar1=float(d * 2 * Wf),
                                scalar2=None, op0=ALU.add)
            idx16 = pool.tile([P, NT * 48], I16)
            v.tensor_copy(out=idx16[0:32], in_=idxf16[0:32])

            # xC build DMAs on scalar, split into quarters: shorter-lived DMA
            # semaphores, so when the early gathers wrap the HWDGE sem pool
            # onto a build they only wait for an early quarter (~10us), not a
            # 40us half.
            Q4 = XC_ROWS // 4
            for m in (0, 1):
                for q in range(4):
                    r0 = q * Q4
                    eng = nc.sync if (m * 4 + q) % 2 else nc.scalar
                    eng.dma_start(
                        out=AP(tensor=xc_t, offset=r0 * 2 * C + m * C,
                               ap=[[2 * C, Q4], [1, C]]),
                        in_=AP(tensor=x_d.tensor, offset=(r0 + 160 * m) * C,
                               ap=[[C, Q4], [1, C]]),
                    )

            # ---------------- shared floor/clamp chain (128-land) -----------
            ixy128 = pool.tile([P, 64], F32)
            v.tensor_scalar(out=ixy128[:], in0=pos128[:], scalar1=SCALE,
                            scalar2=-0.5, op0=ALU.mult, op1=ALU.add)
            r32b = pool.tile([P, 64], I32)
            v.tensor_copy(out=r32b[:], in_=ixy128[:])
            rfb = pool.tile([P, 64], F32)
            v.tensor_copy(out=rfb[:], in_=r32b[:])
            gtb = pool.tile([P, 64], F32)
            v.tensor_tensor(out=gtb[:], in0=rfb[:], in1=ixy128[:], op=ALU.is_gt)
            flb = pool.tile([P, 64], F32)
            v.tensor_tensor(out=flb[:], in0=rfb[:], in1=gtb[:], op=ALU.subtract)
            sfb = pool.tile([P, 64], F32)  # clamp(floor-1, 0, 156)
            v.tensor_scalar(out=sfb[:], in0=flb[:], scalar1=-1.0,
                            scalar2=0.0, op0=ALU.add, op1=ALU.max)
            v.tensor_scalar(out=sfb[:], in0=sfb[:], scalar1=float(Wf - 4),
                            scalar2=None, op0=ALU.min)


            # ---------------- weights (128-land) -----------------------------
            ew = pool.tile([P, 64], F32)  # s - i  (x at even cols, y at odd)
            v.tensor_tensor(out=ew[:], in0=sfb[:], in1=ixy128[:], op=ALU.subtract)

            # d tile [128, 256]: col = xy*128 + T*4 + k ; d = s + k - i
            dte = pool.tile([P, 256], F32)
            e_v = ew[:].rearrange("p (t c) -> p c t", c=2)  # [128, 2, 32]
            for k in range(4):
                outv = dte[:].rearrange("p (c t k) -> p c t k",
                                        t=NT, k=4)[:, :, :, k]
                v.tensor_scalar(out=outv, in0=e_v, scalar1=float(k),
                                scalar2=None, op0=ALU.add)

            # branchless cubic kernel W(d), masked to |d|<2
            av = pool.tile([P, 256], F32)
            v.tensor_scalar(out=av[:], in0=dte[:], scalar1=-1.0, scalar2=None,
                            op0=ALU.mult)
            v.tensor_tensor(out=av[:], in0=av[:], in1=dte[:], op=ALU.max)
            a2 = pool.tile([P, 256], F32)
            v.tensor_tensor(out=a2[:], in0=av[:], in1=av[:], op=ALU.mult)
            t1 = pool.tile([P, 256], F32)  # ((A+2)a - (A+3)) * a^2   (= w_in - 1)
            v.tensor_scalar(out=t1[:], in0=av[:], scalar1=A + 2.0,
                            scalar2=-(A + 3.0), op0=ALU.mult, op1=ALU.add)
            v.tensor_tensor(out=t1[:], in0=t1[:], in1=a2[:], op=ALU.mult)
            u = pool.tile([P, 256], F32)   # ((A a - 5A) a + 8A) a - 4A  (= w_out)
            v.tensor_scalar(out=u[:], in0=av[:], scalar1=A,
                            scalar2=-5.0 * A, op0=ALU.mult, op1=ALU.add)
            v.tensor_tensor(out=u[:], in0=u[:], in1=av[:], op=ALU.mult)
            v.tensor_scalar(out=u[:], in0=u[:], scalar1=8.0 * A,
                            scalar2=None, op0=ALU.add)
            v.tensor_tensor(out=u[:], in0=u[:], in1=av[:], op=ALU.mult)
            v.tensor_scalar(out=u[:], in0=u[:], scalar1=-4.0 * A,
                            scalar2=None, op0=ALU.add)
            m_in = pool.tile([P, 256], F32)
            v.tensor_scalar(out=m_in[:], in0=av[:], scalar1=1.0,
                            scalar2=None, op0=ALU.is_le)
            m_lt2 = pool.tile([P, 256], F32)
            v.tensor_scalar(out=m_lt2[:], in0=av[:], scalar1=2.0,
                            scalar2=None, op0=ALU.is_lt)
            wM = pool.tile([P, 256], F32)
            v.tensor_tensor(out=wM[:], in0=t1[:], in1=u[:], op=ALU.subtract)
            v.tensor_scalar(out=wM[:], in0=wM[:], scalar1=1.0,
                            scalar2=None, op0=ALU.add)       # = w_in - w_out
            v.tensor_tensor(out=wM[:], in0=wM[:], in1=m_in[:], op=ALU.mult)
            v.tensor_tensor(out=wM[:], in0=wM[:], in1=u[:], op=ALU.add)
            v.tensor_tensor(out=wM[:], in0=wM[:], in1=m_lt2[:], op=ALU.mult)
            # wxx = wM[:, T*4+k], wyy = wM[:, 128 + T*4+it]

            # zero tile: STT (scalar*in0 + 0) replaces tensor_scalar with a
            # PTR-sourced scalar for the accumulator init — the PTR mult/bypass
            # form sporadically stalls 17-26us on HW; STT never does.
            zt = pool.tile([P, 4 * C], F32)
            v.memset(zt[:], 0.0)

            # ---------------- gather + reduce ---------------------------------
            def reduce_and_write(gout, off, nt, phase):
                outC = gpool.tile([P, MAXT * C], F32, tag="outC")
                for tl in range(nt):
                    T = off + tl
                    acc = gpool.tile([P, 4 * C], F32, tag="acc")
                    if phase == 1:
                        # gout slot a = tl*4+it, elem [k:4][c:64]; y-reduce
                        # first (contiguous), then x-reduce.
                        for it in range(4):
                            src = gout[:, (tl * 4 + it) * 4 * C:
                                       (tl * 4 + it + 1) * 4 * C]
                            wy_s = wM[:, 128 + T * 4 + it:128 + T * 4 + it + 1]
                            v.scalar_tensor_tensor(
                                out=acc[:], in0=src, scalar=wy_s,
                                in1=zt[:] if it == 0 else acc[:],
                                op0=ALU.mult, op1=ALU.add)
                        wsel = 0  # x-weights on second loop
                    else:
                        # gout slot a = tl*2+d, elem [k:4][m:2][c:64] where
                        # y-tap i = 2d+m; x-reduce first: acc[(d,m,c)] +=
                        # wx_k * gout[:, 2tl:2tl+2, k-block]
                        gv = gout[:].rearrange("p (a e) -> p a e", e=8 * C)
                        for k in range(4):
                            src = gv[:, 2 * tl:2 * tl + 2,
                                     k * 2 * C:(k + 1) * 2 * C]
                            wx_s = wM[:, T * 4 + k:T * 4 + k + 1]
                            v.scalar_tensor_tensor(
                                out=acc[:].rearrange("p (a e) -> p a e",
                                                     a=2),
                                in0=src, scalar=wx_s,
                                in1=(zt[:] if k == 0 else acc[:]).rearrange(
                                    "p (a e) -> p a e", a=2),
                                op0=ALU.mult, op1=ALU.add)
                        wsel = 128  # y-weights on second loop
                    for k in range(4):
                        src = acc[:, k * C:(k + 1) * C]
                        w_s = wM[:, wsel + T * 4 + k:wsel + T * 4 + k + 1]
                        dst = outC[:, tl * C:(tl + 1) * C]
                        v.scalar_tensor_tensor(
                            out=dst, in0=src, scalar=w_s,
                            in1=zt[:, :C] if k == 0 else dst,
                            op0=ALU.mult, op1=ALU.add)
                # out[p*32 + off + tl, c] — contiguous (tl, c) per partition
                nc.sync.dma_start(
                    out=AP(tensor=out_d.tensor, offset=(off * C),
                           ap=[[NT * C, P], [C, nt], [1, C]]),
                    in_=outC[:, :nt * C].rearrange("p (t c) -> p t c", c=C),
                )

            for _ in range(iters):
                off = 0
                for nt in CHUNKS1:
                    gout = gpool.tile([P, MAXT * 4 * 4 * C], F32, tag="gout")
                    nc.gpsimd.dma_gather(
                        out_ap=gout[:, :nt * 4 * 4 * C].rearrange(
                            "p (g e) -> p g e", e=4 * C),
                        in_ap=x_src,
                        idxs_ap=idx16[:, off * 32:(off + nt) * 32],
                        num_idxs=nt * 4 * P,
                        num_idxs_reg=nt * 4 * P,
                        elem_size=4 * C,
                        elem_step=C,
                        single_packet=False,
                    )
                    reduce_and_write(gout, off, nt, 1)
                    off += nt
                for nt in CHUNKS2:
                    gout = gpool.tile([P, MAXT * 4 * 4 * C], F32, tag="gout")
                    nc.gpsimd.dma_gather(
                        out_ap=gout[:, :nt * 2 * 8 * C].rearrange(
                            "p (g e) -> p g e", e=8 * C),
                        in_ap=xc_src,
                        idxs_ap=idx16[:, NT * 32 + off * 16:
                                      NT * 32 + (off + nt) * 16],
                        num_idxs=nt * 2 * P,
                        num_idxs_reg=nt * 2 * P,
                        elem_size=8 * C,
                        elem_step=2 * C,
                        single_packet=False,
                    )
                    reduce_and_write(gout, off, nt, 2)
                    off += nt
    nc.compile()
    return nc


_NC = None


def _get_nc():
    global _NC
    if _NC is None:
        _NC = build_nc(B)
    return _NC


def kernel(x, pos, H=None, W=None):
    x = np.asarray(x, dtype=np.float32)
    pos = np.asarray(pos, dtype=np.float32)
    assert x.shape == (B, Hf, Wf, C) and pos.shape == (B, N, 2)
    nc = _get_nc()
    in_maps = [
        {"x": np.ascontiguousarray(x[b].reshape(NPIX, C)),
         "pos": np.ascontiguousarray(pos[b])}
        for b in range(B)
    ]
    res = bass_utils.run_bass_kernel_spmd(nc, in_maps, core_ids=list(range(B)))
    return np.stack([res.results[b]["out"] for b in range(B)])


# revision 47
# speedup vs baseline: 1.0129x; 1.0129x over previous
"""Bicubic sparse grid_sample (InterpolateSparse2d) for Trainium2.

Strategy: data-parallel over batch (8 batches -> 8 NeuronCores).
Per core:
  - compute clamped 4x4 patch window start (sx, sy) per point on-device
  - SWDGE dma_gather of the 4x4x64 patches; GpSimd descriptor generation
    (~8ns/desc on 2 Q7 cores) is the bottleneck, so descriptors per point
    are cut from 4 to 2 for most tiles via xC, a row-pair-interleaved HBM
    copy of x (xC[r] = [x[r] | x[r+160]]) built on the fly by HBM->HBM
    DMAs on the otherwise-idle scalar HWDGE queue: one 2KB element at xC
    row sy*160+sx covers patch rows (sy, sy+1), row +320 covers (sy+2,
    sy+3). Early tiles (phase 1) gather 4-desc straight from x while the
    build is in flight; later tiles (phase 2) gather 2-desc from xC.
  - cubic weights W(d) evaluated branchlessly per fetched slot; OOB taps
    get |d|>=2 -> weight 0 (matches zeros-padding + ref masking)
  - per-partition-scalar multiply-accumulate reduce on VectorE
    (scalar_tensor_tensor only: tensor_scalar with a PTR scalar hits a
    sporadic 17-26us HW slow path)

Point-to-partition mapping is n = p*32 + t (32 consecutive points per
partition) so the pos loads (128-land for weights, 16-land for gather
indices) and the output writeback are all contiguous-chunk DMAs.  Gather
chunks are small-first (quick first MAC) and tiny-last (short tail).
"""

import numpy as np

import concourse.bacc as bacc
import concourse.mybir as mybir
import concourse.tile as tile
from concourse import bass_utils
from concourse.ap import AP

F32 = mybir.dt.float32
I32 = mybir.dt.int32
I16 = mybir.dt.int16
ALU = mybir.AluOpType
AF = mybir.ActivationFunctionType

B, Hf, Wf, C = 8, 160, 160, 64
N = 4096
H, W = 1280, 1280
A = -0.75
SCALE = float(Wf) / float(W - 1)  # == Hf/(H-1); maps pixel coords -> feature coords
P = 128
NT = N // P          # 32 point-tiles of 128
# phase 1: 4-desc gathers straight from x while xC builds in background;
# phase 2: 2-desc gathers from the row-pair-interleaved xC copy.
CHUNKS1 = [1, 1, 2, 2, 2, 2]  # tiles per gather, phase 1
CHUNKS2 = [3, 4, 4, 4, 3, 2, 1, 1]  # tiles per gather, phase 2
NPIX = Hf * Wf       # 25600
# gather source AP row count: last valid row start is 156*160+156+480 = 25596,
# and (rows-1)*64 + 256 must stay inside the x tensor (25600*64 elements)
SRC_ROWS = NPIX - 3  # 25597
XC_ROWS = 25440      # xC rows actually referenced: r0+323 <= 25439
MAXT = 8


def build_nc(num_devices: int, iters: int = 1):
    nc = bacc.Bacc(
        "TRN2", target_bir_lowering=False, debug=False,
        enable_asserts=False, num_devices=num_devices,
    )
    x_d = nc.dram_tensor("x", [NPIX, C], F32, kind="ExternalInput").ap()
    pos_d = nc.dram_tensor("pos", [N, 2], F32, kind="ExternalInput").ap()
    out_d = nc.dram_tensor("out", [N, C], F32, kind="ExternalOutput").ap()

    x_src = AP(tensor=x_d.tensor, offset=0, ap=[[C, SRC_ROWS], [1, 4 * C]])

    with tile.TileContext(nc) as tc:
        with tc.tile_pool(name="sbuf", bufs=1) as pool, \
             tc.tile_pool(name="dpool", bufs=1, space="DRAM") as dpool, \
             tc.tile_pool(name="gpool", bufs=4) as gpool:
            v = nc.vector

            # ------- xC: row-pair interleave in HBM ------------------------
            # xC[r] = [x[r] | x[r+160]]; a 4-row gather element at xC row r0
            # covers the 4x4 patch rows (sy, sy+1); r0+320 covers (sy+2, sy+3).
            # Built below (after the idx-path DMAs are queued) by pure
            # HBM->HBM DMAs on the sync/scalar HWDGE queues; the 12720-desc
            # issues overlap the phase-1 gathers.
            xc = dpool.tile([NPIX, 2 * C], F32)
            xc_t = xc[:].tensor
            xc_src = AP(tensor=xc_t, offset=0,
                        ap=[[2 * C, XC_ROWS - 3], [1, 4 * 2 * C]])

            # ---------------- pos load: pos128[p, s*2+c] = pos[p*32+s, c] ----
            # contiguous 256B per partition
            pos128 = pool.tile([P, NT * 2], F32)
            nc.sync.dma_start(
                out=pos128[:].rearrange("p (s c) -> p s c", c=2),
                in_=pos_d.rearrange("(p s) c -> p s c", p=P),
            )

            # ---------------- gather indices (16-partition land) -------------
            # The idx tile must live in partitions 0..31 as [j, col] with
            # col = (slot)*8 + ph for point p = ph*16+j.  Computing the chain
            # directly in 16-partition land (from a second pos load laid out
            # [j, (h, t, c)]) avoids any 128->16 transpose DMAs, keeping the
            # early HWDGE DMA count below the sem-pool size.
            pos16 = pool.tile([P, 512], F32)
            for jb in (0, 16):
                nc.sync.dma_start(
                    out=pos16[jb:jb + 16].rearrange("j (h t c) -> j h t c",
                                                    h=8, c=2),
                    in_=pos_d.rearrange("(h j t) c -> j h t c", h=8, j=16),
                )
            ixy16 = pool.tile([P, 512], F32)
            v.tensor_scalar(out=ixy16[0:32], in0=pos16[0:32], scalar1=SCALE,
                            scalar2=-0.5, op0=ALU.mult, op1=ALU.add)
            r16 = pool.tile([P, 512], I32)
            v.tensor_copy(out=r16[0:32], in_=ixy16[0:32])
            rf16 = pool.tile([P, 512], F32)
            v.tensor_copy(out=rf16[0:32], in_=r16[0:32])
            gt16 = pool.tile([P, 512], F32)
            v.tensor_tensor(out=gt16[0:32], in0=rf16[0:32],
                            in1=ixy16[0:32], op=ALU.is_gt)
            fl16 = pool.tile([P, 512], F32)
            v.tensor_tensor(out=fl16[0:32], in0=rf16[0:32],
                            in1=gt16[0:32], op=ALU.subtract)
            sf16 = pool.tile([P, 512], F32)
            v.tensor_scalar(out=sf16[0:32], in0=fl16[0:32], scalar1=-1.0,
                            scalar2=0.0, op0=ALU.add, op1=ALU.max)
            v.tensor_scalar(out=sf16[0:32], in0=sf16[0:32],
                            scalar1=float(Wf - 4), scalar2=None, op0=ALU.min)
            # base16[j, h*32+t] = sy*160 + sx for point (h*16+j)*32 + t
            base16 = pool.tile([P, NT * 8], F32)
            sxy16 = sf16[0:32].rearrange("j (h t c) -> j (h t) c", h=8, c=2)
            v.tensor_scalar(out=base16[0:32], in0=sxy16[:, :, 1],
                                    scalar1=float(Wf), scalar2=None,
                                    op0=ALU.mult)
            v.tensor_tensor(out=base16[0:32], in0=base16[0:32],
                                    in1=sxy16[:, :, 0], op=ALU.add)
            # phase-1 idx cols [0, NT*32): [j, t*32 + it*8 + ph] = base + 160*it
            # phase-2 idx cols [NT*32, NT*48): [j, t*16 + d*8 + ph] = base + 320*d
            idxf16 = pool.tile([P, NT * 48], F32)
            bv = base16[0:32].rearrange("j (h t) -> j t h", t=NT)
            for it in range(4):
                outv = idxf16[0:32, :NT * 32].rearrange(
                    "j (t i h) -> j t i h", i=4, h=8)[:, :, it, :]
                v.tensor_scalar(out=outv, in0=bv, scalar1=float(it * Wf),
                                scalar2=None, op0=ALU.add)
            for d in range(2):
                outv = idxf16[0:32, NT * 32:].rearrange(
                    "j (t i h) -> j t i h", i=2, h=8)[:, :, d, :]
                v.tensor_scalar(out=outv, in0=bv, scalar1=float(d * 2 * Wf),
                                scalar2=None, op0=ALU.add)
            idx16 = pool.tile([P, NT * 48], I16)
            v.tensor_copy(out=idx16[0:32], in_=idxf16[0:32])

            # xC build DMAs on scalar, split into quarters: shorter-lived DMA
            # semaphores, so when the early gathers wrap the HWDGE sem pool
            # onto a build they only wait for an early quarter (~10us), not a
            # 40us half.
            Q4 = XC_ROWS // 4
            for m in (0, 1):
                for q in range(4):
                    r0 = q * Q4
                    eng = nc.sync if (m * 4 + q) % 2 else nc.scalar
                    eng.dma_start(
                        out=AP(tensor=xc_t, offset=r0 * 2 * C + m * C,
                               ap=[[2 * C, Q4], [1, C]]),
                        in_=AP(tensor=x_d.tensor, offset=(r0 + 160 * m) * C,
                               ap=[[C, Q4], [1, C]]),
                    )

            # ---------------- shared floor/clamp chain (128-land) -----------
            ixy128 = pool.tile([P, 64], F32)
            v.tensor_scalar(out=ixy128[:], in0=pos128[:], scalar1=SCALE,
                            scalar2=-0.5, op0=ALU.mult, op1=ALU.add)
            r32b = pool.tile([P, 64], I32)
            v.tensor_copy(out=r32b[:], in_=ixy128[:])
            rfb = pool.tile([P, 64], F32)
            v.tensor_copy(out=rfb[:], in_=r32b[:])
            gtb = pool.tile([P, 64], F32)
            v.tensor_tensor(out=gtb[:], in0=rfb[:], in1=ixy128[:], op=ALU.is_gt)
            flb = pool.tile([P, 64], F32)
            v.tensor_tensor(out=flb[:], in0=rfb[:], in1=gtb[:], op=ALU.subtract)
            sfb = pool.tile([P, 64], F32)  # clamp(floor-1, 0, 156)
            v.tensor_scalar(out=sfb[:], in0=flb[:], scalar1=-1.0,
                            scalar2=0.0, op0=ALU.add, op1=ALU.max)
            v.tensor_scalar(out=sfb[:], in0=sfb[:], scalar1=float(Wf - 4),
                            scalar2=None, op0=ALU.min)


            # ---------------- weights (128-land) -----------------------------
            ew = pool.tile([P, 64], F32)  # s - i  (x at even cols, y at odd)
            v.tensor_tensor(out=ew[:], in0=sfb[:], in1=ixy128[:], op=ALU.subtract)

            # d tile [128, 256]: col = xy*128 + T*4 + k ; d = s + k - i
            dte = pool.tile([P, 256], F32)
            e_v = ew[:].rearrange("p (t c) -> p c t", c=2)  # [128, 2, 32]
            for k in range(4):
                outv = dte[:].rearrange("p (c t k) -> p c t k",
                                        t=NT, k=4)[:, :, :, k]
                v.tensor_scalar(out=outv, in0=e_v, scalar1=float(k),
                                scalar2=None, op0=ALU.add)

            # branchless cubic kernel W(d), masked to |d|<2
            av = pool.tile([P, 256], F32)
            v.tensor_scalar(out=av[:], in0=dte[:], scalar1=-1.0, scalar2=None,
                            op0=ALU.mult)
            v.tensor_tensor(out=av[:], in0=av[:], in1=dte[:], op=ALU.max)
            a2 = pool.tile([P, 256], F32)
            v.tensor_tensor(out=a2[:], in0=av[:], in1=av[:], op=ALU.mult)
            t1 = pool.tile([P, 256], F32)  # ((A+2)a - (A+3)) * a^2   (= w_in - 1)
            v.tensor_scalar(out=t1[:], in0=av[:], scalar1=A + 2.0,
                            scalar2=-(A + 3.0), op0=ALU.mult, op1=ALU.add)
            v.tensor_tensor(out=t1[:], in0=t1[:], in1=a2[:], op=ALU.mult)
            u = pool.tile([P, 256], F32)   # ((A a - 5A) a + 8A) a - 4A  (= w_out)
            v.tensor_scalar(out=u[:], in0=av[:], scalar1=A,
                            scalar2=-5.0 * A, op0=ALU.mult, op1=ALU.add)
            v.tensor_tensor(out=u[:], in0=u[:], in1=av[:], op=ALU.mult)
            v.tensor_scalar(out=u[:], in0=u[:], scalar1=8.0 * A,
                            scalar2=None, op0=ALU.add)
            v.tensor_tensor(out=u[:], in0=u[:], in1=av[:], op=ALU.mult)
            v.tensor_scalar(out=u[:], in0=u[:], scalar1=-4.0 * A,
                            scalar2=None, op0=ALU.add)
            m_in = pool.tile([P, 256], F32)
            v.tensor_scalar(out=m_in[:], in0=av[:], scalar1=1.0,
                            scalar2=None, op0=ALU.is_le)
            m_lt2 = pool.tile([P, 256], F32)
            v.tensor_scalar(out=m_lt2[:], in0=av[:], scalar1=2.0,
                            scalar2=None, op0=ALU.is_lt)
            wM = pool.tile([P, 256], F32)
            v.tensor_tensor(out=wM[:], in0=t1[:], in1=u[:], op=ALU.subtract)
            v.tensor_scalar(out=wM[:], in0=wM[:], scalar1=1.0,
                            scalar2=None, op0=ALU.add)       # = w_in - w_out
            v.tensor_tensor(out=wM[:], in0=wM[:], in1=m_in[:], op=ALU.mult)
            v.tensor_tensor(out=wM[:], in0=wM[:], in1=u[:], op=ALU.add)
            v.tensor_tensor(out=wM[:], in0=wM[:], in1=m_lt2[:], op=ALU.mult)
            # wxx = wM[:, T*4+k], wyy = wM[:, 128 + T*4+it]

            # zero tile: STT (scalar*in0 + 0) replaces tensor_scalar with a
            # PTR-sourced scalar for the accumulator init — the PTR mult/bypass
            # form sporadically stalls 17-26us on HW; STT never does.
            zt = pool.tile([P, 4 * C], F32)
            v.memset(zt[:], 0.0)

            # ---------------- gather + reduce ---------------------------------
            def reduce_and_write(gout, off, nt, phase):
                outC = gpool.tile([P, MAXT * C], F32, tag="outC")
                for tl in range(nt):
                    T = off + tl
                    acc = gpool.tile([P, 4 * C], F32, tag="acc")
                    if phase == 1:
                        # gout slot a = tl*4+it, elem [k:4][c:64]; y-reduce
                        # first (contiguous), then x-reduce.
                        for it in range(4):
                            src = gout[:, (tl * 4 + it) * 4 * C:
                                       (tl * 4 + it + 1) * 4 * C]
                            wy_s = wM[:, 128 + T * 4 + it:128 + T * 4 + it + 1]
                            v.scalar_tensor_tensor(
                                out=acc[:], in0=src, scalar=wy_s,
                                in1=zt[:] if it == 0 else acc[:],
                                op0=ALU.mult, op1=ALU.add)
                        wsel = 0  # x-weights on second loop
                    else:
                        # gout slot a = tl*2+d, elem [k:4][m:2][c:64] where
                        # y-tap i = 2d+m; x-reduce first: acc[(d,m,c)] +=
                        # wx_k * gout[:, 2tl:2tl+2, k-block]
                        gv = gout[:].rearrange("p (a e) -> p a e", e=8 * C)
                        for k in range(4):
                            src = gv[:, 2 * tl:2 * tl + 2,
                                     k * 2 * C:(k + 1) * 2 * C]
                            wx_s = wM[:, T * 4 + k:T * 4 + k + 1]
                            v.scalar_tensor_tensor(
                                out=acc[:].rearrange("p (a e) -> p a e",
                                                     a=2),
                                in0=src, scalar=wx_s,
                                in1=(zt[:] if k == 0 else acc[:]).rearrange(
                                    "p (a e) -> p a e", a=2),
                                op0=ALU.mult, op1=ALU.add)
                        wsel = 128  # y-weights on second loop
                    for k in range(4):
                        src = acc[:, k * C:(k + 1) * C]
                        w_s = wM[:, wsel + T * 4 + k:wsel + T * 4 + k + 1]
                        dst = outC[:, tl * C:(tl + 1) * C]
                        v.scalar_tensor_tensor(
                            out=dst, in0=src, scalar=w_s,
                            in1=zt[:, :C] if k == 0 else dst,
                            op0=ALU.mult, op1=ALU.add)
                # out[p*32 + off + tl, c] — contiguous (tl, c) per partition
                nc.sync.dma_start(
                    out=AP(tensor=out_d.tensor, offset=(off * C),
                           ap=[[NT * C, P], [C, nt], [1, C]]),
                    in_=outC[:, :nt * C].rearrange("p (t c) -> p t c", c=C),
                )

            for _ in range(iters):
                off = 0
                for nt in CHUNKS1:
                    gout = gpool.tile([P, MAXT * 4 * 4 * C], F32, tag="gout")
                    nc.gpsimd.dma_gather(
                        out_ap=gout[:, :nt * 4 * 4 * C].rearrange(
                            "p (g e) -> p g e", e=4 * C),
                        in_ap=x_src,
                        idxs_ap=idx16[:, off * 32:(off + nt) * 32],
                        num_idxs=nt * 4 * P,
                        num_idxs_reg=nt * 4 * P,
                        elem_size=4 * C,
                        elem_step=C,
                        single_packet=False,
                    )
                    reduce_and_write(gout, off, nt, 1)
                    off += nt
                for nt in CHUNKS2:
                    gout = gpool.tile([P, MAXT * 4 * 4 * C], F32, tag="gout")
                    nc.gpsimd.dma_gather(
                        out_ap=gout[:, :nt * 2 * 8 * C].rearrange(
                            "p (g e) -> p g e", e=8 * C),
                        in_ap=xc_src,
                        idxs_ap=idx16[:, NT * 32 + off * 16:
                                      NT * 32 + (off + nt) * 16],
                        num_idxs=nt * 2 * P,
                        num_idxs_reg=nt * 2 * P,
                        elem_size=8 * C,
                        elem_step=2 * C,
                        single_packet=False,
                    )
                    reduce_and_write(gout, off, nt, 2)
                    off += nt
    nc.compile()
    return nc


_NC = None


def _get_nc():
    global _NC
    if _NC is None:
        _NC = build_nc(B)
    return _NC


def kernel(x, pos, H=None, W=None):
    x = np.asarray(x, dtype=np.float32)
    pos = np.asarray(pos, dtype=np.float32)
    assert x.shape == (B, Hf, Wf, C) and pos.shape == (B, N, 2)
    nc = _get_nc()
    in_maps = [
        {"x": np.ascontiguousarray(x[b].reshape(NPIX, C)),
         "pos": np.ascontiguousarray(pos[b])}
        for b in range(B)
    ]
    res = bass_utils.run_bass_kernel_spmd(nc, in_maps, core_ids=list(range(B)))
    return np.stack([res.results[b]["out"] for b in range(B)])
